# revision 27
# baseline (speedup 1.0000x reference)
"""BitNetLinear on 8 Trainium2 NeuronCores.

Computes out = x @ sign(weight).T + bias for x[4,2048,4096] f32,
weight[4096,4096] f32, bias[4096] f32.

Strategy: 2-way data parallel over rows x 4-way tensor parallel over
out_features (each core owns a [4096, 1024] block of the [8192, 4096]
output; no collectives, host stitches blocks).

Per core the contraction (4096 = 32 blocks of 128) is mixed-precision:
  - k-blocks [0, G)   : x quantized e4m3, fp8 DoubleRow matmuls
                        (k=256/instr, ~241 ns at N=512);
  - k-blocks [G, 32)  : x in fp16, normal matmuls (~213 ns at N=512).
sign(weight) is exact in both dtypes; PSUM accumulates fp32. The only
error source is e4m3 quantization of x on the G fp8 blocks: measured
rel-l2 = 1.33e-2 at G=8 on the benchmark inputs (fp16-only is 2.1e-4,
fp8-only 2.65e-2). G trades speed vs accuracy; G=8 keeps 34% margin
to the 2e-2 gate and runs ~1.75x faster than the fp16+fp8 hi/lo
baseline (DoubleRow costs 1.13x a fp16 matmul per instruction, so a
full hi/lo split is slower than plain fp16; raw-fp8 blocks are the
only way below one fp16 pass).

Layouts are precomputed on the host so every DMA is contiguous. All
weights stay resident in SBUF, x tiles stream per m-tile, and each
[128, 512] output chunk accumulates (32-G) fp16 + G/2 DoubleRow
matmuls before a DVE eviction fused with the bias add. The first
three m-tiles run jointly, k-block-major, so PE consumption paces the
weight preload instead of stalling on it. Steady-state m-tile pairs
alternate block order (fp16,fp16,fp8,fp8 | fp8,fp8,fp16,fp16) so
weight-path mode switches drop to one per two m-tiles.
"""

import sys
import types

import numpy as np

import concourse.mybir as mybir
import concourse.tile as tile
from concourse import bacc
from concourse.bass_utils import run_bass_kernel_spmd


def _ensure_axon_hooks():
    """run_bass_kernel_spmd(trace=True) (or BASS_TRACE=1 in the env) imports
    antenv.axon_hooks, which some agent images lack. Provide it, and register
    the ctypes NTFF hook if the boot shim is available, so tracing works (or
    degrades to a warning) instead of crashing."""
    try:
        import antenv.axon_hooks  # noqa: F401

        return
    except ImportError:
        pass
    m = types.ModuleType("antenv.axon_hooks")
    m._h = None
    m.set_axon_ntff_profile_hook = lambda h: setattr(m, "_h", h)
    m.get_axon_ntff_profile_hook = lambda: m._h
    sys.modules["antenv.axon_hooks"] = m
    try:
        import antenv

        antenv.axon_hooks = m
    except ImportError:
        pass
    try:
        from trn_agent_boot.trn_boot import _ntff_profile_via_ctypes

        m.set_axon_ntff_profile_hook(
            _ntff_profile_via_ctypes("/opt/axon/libaxon_pjrt.so")
        )
    except Exception:
        pass


_ensure_axon_hooks()

B, S, D_IN, D_OUT = 4, 2048, 4096, 4096
M_TOT = B * S  # 8192
N_CORES = 8
MG, OG = 2, 4  # data-parallel row groups x tensor-parallel out_feature groups
M_SH = M_TOT // MG  # 4096 rows per core
O_SH = D_OUT // OG  # 1024 out features per core
P = 128
DB = D_IN // P  # 32 contraction blocks of 128
G = 12  # k-blocks [0, G) in e4m3 DoubleRow; must be even
GP = G // 2  # fp8 contraction pairs of 256 (DoubleRow)
LB = DB - G  # fp16 contraction blocks
MT = M_SH // P  # 32 m-tiles per core
NF = 512  # moving free dim per matmul (one PSUM bank of fp32)
NCH = O_SH // NF  # 2 output chunks per m-tile
ST = 3  # m-tiles processed jointly in the startup phase
# startup stream granularity: larger per-partition DMA lines lift the
# per-packet-bound DMA rate (2KB lines measured ~235 GB/s core-wide;
# the startup needs ~280)
XCH = [min(8, LB - 8 * i) for i in range((LB + 7) // 8)]  # xls chunks
# w16 groups: first two are pairs so the fp8->fp16 handoff in the startup
# stream doesn't wait on a full 1MB group
WGS = [2, 2] + [4] * ((LB - 4) // 4)
assert sum(WGS) == LB
_WOFF = [sum(WGS[:i]) for i in range(len(WGS))]  # first lb of each group

_CACHE = {}


def _build():
    nc = bacc.Bacc("TRN2", target_bir_lowering=False, debug=False)
    f8, f16, f32 = mybir.dt.float8e4, mybir.dt.float16, mybir.dt.float32

    # steady-state x, one m-tile per row: free = dp*256 + h*128 + m (fp8)
    # and lb*128 + m (fp16)
    xh_d = nc.dram_tensor("xh", [MT, P, G * P], f8, kind="ExternalInput")
    xl_d = nc.dram_tensor("xl", [MT, P, LB * P], f16, kind="ExternalInput")
    # startup copies of m-tiles 0..ST-1, k-block-major: free dim runs over
    # (dp|lb, st, m) so each transfer covers many k-blocks in one DMA with
    # wide per-partition lines
    # dp 0 ships alone so the very first matmul only waits on ~0.36 MB
    xhs0_d = nc.dram_tensor("xhs0", [P, ST * 2 * P], f8, kind="ExternalInput")
    xhs1_d = nc.dram_tensor(
        "xhs1", [P, (GP - 1) * ST * 2 * P], f8, kind="ExternalInput"
    )
    xls_d = [
        nc.dram_tensor(f"xls{c}", [P, n * ST * P], f16, kind="ExternalInput")
        for c, n in enumerate(XCH)
    ]
    # weights: fp8 pair layout per dp, fp16 in groups of 4 k-blocks
    w8_d = nc.dram_tensor("w8", [GP, P, 2 * O_SH], f8, kind="ExternalInput")
    w16_d = [
        nc.dram_tensor(f"w16g{q}", [P, n * O_SH], f16, kind="ExternalInput")
        for q, n in enumerate(WGS)
    ]
    bias_d = nc.dram_tensor("biasb", [P, O_SH], f32, kind="ExternalInput")
    out_d = nc.dram_tensor("out", [M_SH, O_SH], f32, kind="ExternalOutput")

    with tile.TileContext(nc) as tc:
        with (
            tc.tile_pool(name="wpool", bufs=1) as wpool,
            tc.tile_pool(name="xpool", bufs=6) as xpool,
            tc.tile_pool(name="psum", bufs=4, space="PSUM") as psum_pool,
        ):

            def load_x(mt):
                x_hi = xpool.tile([P, G * P], f8, name="x_hi", tag="xhi")
                x_lo = xpool.tile([P, LB * P], f16, name="x_lo", tag="xlo")
                nc.sync.dma_start(out=x_hi[:], in_=xh_d[mt])
                nc.sync.dma_start(out=x_lo[:], in_=xl_d[mt])
                return x_hi, x_lo

            def alloc_psums():
                return [
                    psum_pool.tile([P, NF], f32, name=f"ps{oc}", tag=f"ps{oc}")
                    for oc in range(NCH)
                ]

            def lo_block(x_lo, psums, opens, closes):
                # full fp16 pass over one m-tile; opens/closes the psum
                # accumulation group if it is the first/last block issued
                for lb in range(LB):
                    for oc in range(NCH):
                        nc.tensor.matmul(
                            psums[oc][:],
                            x_lo[:, lb * P : (lb + 1) * P],
                            w16_sb[lb][:, oc * NF : (oc + 1) * NF],
                            start=opens and lb == 0,
                            stop=closes and lb == LB - 1,
                        )

            def hi_block(x_hi, psums, opens, closes):
                # full DoubleRow fp8 pass over one m-tile
                for dp in range(GP):
                    lhsT3 = x_hi[:, dp * 2 * P : (dp + 1) * 2 * P].rearrange(
                        "p (h m) -> p h m", h=2
                    )
                    for oc in range(NCH):
                        nc.tensor.matmul(
                            psums[oc][:],
                            lhsT3,
                            w8_sb[dp][:]
                            .rearrange("p (h o) -> p h o", h=2)[
                                :, :, oc * NF : (oc + 1) * NF
                            ],
                            start=opens and dp == 0,
                            stop=closes and dp == GP - 1,
                            perf_mode=mybir.MatmulPerfMode.DoubleRow,
                        )

            def evict(opool, mt, psums, ocs=None):
                for oc in ocs if ocs is not None else range(NCH):
                    o_sb = opool.tile([P, NF], f32, name="o_sb", tag=f"o{oc}")
                    nc.vector.tensor_add(
                        o_sb[:], psums[oc][:], bias_sb[:, oc * NF : (oc + 1) * NF]
                    )
                    # scalar queue: keeps evictions off the sync queue so
                    # steady x loads never wait behind them
                    nc.scalar.dma_start(
                        out=out_d[mt * P : (mt + 1) * P, oc * NF : (oc + 1) * NF],
                        in_=o_sb[:],
                    )

            w8_sb = []
            w16_sb = []
            with tc.tile_pool(name="xstart", bufs=1) as xstart_pool:
                # startup x (m-tiles 0..ST-1) in k-major order plus the
                # weight stream, interleaved in consumption order so each
                # tile lands as the PE needs it: fp8 phase first, then the
                # fp16 blocks (xls chunk / w16 group issued just before the
                # k-blocks they cover)
                # the first-MM-critical transfers ride the (otherwise idle)
                # scalar queue so they prime in parallel with the sync stream
                xhs0_sb = xstart_pool.tile(
                    [P, ST * 2 * P], f8, name="xhs0", tag="xhs0"
                )
                nc.scalar.dma_start(out=xhs0_sb[:], in_=xhs0_d[:])
                xhs1_sb = xstart_pool.tile(
                    [P, (GP - 1) * ST * 2 * P], f8, name="xhs1", tag="xhs1"
                )
                for dp in range(GP):
                    w8 = wpool.tile(
                        [P, 2 * O_SH], f8, name=f"w8_{dp}", tag=f"w8_{dp}"
                    )
                    (nc.scalar if dp == 0 else nc.sync).dma_start(
                        out=w8[:], in_=w8_d[dp]
                    )
                    w8_sb.append(w8)
                    if dp == 0:
                        nc.sync.dma_start(out=xhs1_sb[:], in_=xhs1_d[:])
                xls_view = []  # per lb: AP covering [P, ST*P]
                q = -1
                for lb in range(LB):
                    if lb % 8 == 0:
                        c = lb // 8
                        xc = xstart_pool.tile(
                            [P, XCH[c] * ST * P], f16, name=f"xls{c}",
                            tag=f"xls{c}",
                        )
                        nc.sync.dma_start(out=xc[:], in_=xls_d[c][:])
                    if q + 1 < len(WGS) and lb == _WOFF[q + 1]:
                        q += 1
                        wg = wpool.tile(
                            [P, WGS[q] * O_SH], f16, name=f"w16g{q}",
                            tag=f"w16g{q}",
                        )
                        nc.sync.dma_start(out=wg[:], in_=w16_d[q][:])
                    xls_view.append(
                        xc[:, (lb % 8) * ST * P : (lb % 8 + 1) * ST * P]
                    )
                    j = lb - _WOFF[q]
                    w16_sb.append(wg[:, j * O_SH : (j + 1) * O_SH])
                bias_sb = wpool.tile([P, O_SH], f32, name="bias_sb")
                nc.sync.dma_start(out=bias_sb[:], in_=bias_d[:])

                # prefetch steady-state x ahead of the startup evictions
                # (in-order sync stream: later dma_starts would head-of-line
                # block behind eviction DMAs otherwise)
                x_next = {mt: load_x(mt) for mt in (ST, ST + 1)}

                # startup: ST m-tiles jointly, k-major, paced by the weight
                # stream
                psums_st = [alloc_psums() for _ in range(ST)]
                for dp in range(GP):
                    src = xhs0_sb if dp == 0 else xhs1_sb
                    base = 0 if dp == 0 else (dp - 1) * ST
                    for st in range(ST):
                        xh3 = src[
                            :, (base + st) * 2 * P : (base + st + 1) * 2 * P
                        ].rearrange("p (h m) -> p h m", h=2)
                        for oc in range(NCH):
                            nc.tensor.matmul(
                                psums_st[st][oc][:],
                                xh3,
                                w8_sb[dp][:]
                                .rearrange("p (h o) -> p h o", h=2)[
                                    :, :, oc * NF : (oc + 1) * NF
                                ],
                                start=dp == 0,
                                stop=False,
                                perf_mode=mybir.MatmulPerfMode.DoubleRow,
                            )
                for lb in range(LB):
                    for st in range(ST):
                        for oc in range(NCH):
                            nc.tensor.matmul(
                                psums_st[st][oc][:],
                                xls_view[lb][:, st * P : (st + 1) * P],
                                w16_sb[lb][:, oc * NF : (oc + 1) * NF],
                                start=False,
                                stop=lb == LB - 1,
                            )

            with tc.tile_pool(name="opool", bufs=2) as opool:
                for st in range(ST):
                    evict(opool, st, psums_st[st])

                # Steady state: groups of m-tiles with alternating block order
                # (lo,..,hi,.. | hi,..,lo,.. | ...) so fp16<->DoubleRow
                # weight-path mode switches drop to one per group (the group
                # boundary joins identical modes). The startup ends on a fp16
                # matmul, so the first group opens lo. Group size 3 holds
                # 3 psum gens (6 banks) live, within the 4-gen pool.
                # First group is a pair: its 2nd psum gen recycles a startup
                # gen, and the smaller group keeps that wait off the critical
                # path right at the transition.
                sizes = [2] + [3] * ((MT - 1 - ST - 4) // 3) + [2]
                assert sum(sizes) == MT - 1 - ST
                groups = []
                t = ST
                for n in sizes:
                    groups.append(tuple(range(t, t + n)))
                    t += n
                for pi_, grp in enumerate(groups):
                    xs = [
                        x_next.pop(m) if m in x_next else load_x(m)
                        for m in grp
                    ]
                    pss = [alloc_psums() for _ in grp]
                    ii = range(len(grp))
                    if pi_ % 2 == 0:
                        for i in ii:
                            lo_block(xs[i][1], pss[i], True, False)
                        for i in ii:
                            hi_block(xs[i][0], pss[i], False, True)
                    else:
                        for i in ii:
                            hi_block(xs[i][0], pss[i], True, False)
                        for i in ii:
                            lo_block(xs[i][1], pss[i], False, True)
                    for i in ii:
                        evict(opool, grp[i], pss[i])
                for mt in (MT - 1,):
                    # last m-tile: oc-major so each output chunk finishes
                    # and evicts as early as possible
                    x_pair = x_next.pop(mt) if mt in x_next else load_x(mt)
                    x_hi, x_lo = x_pair
                    psums = alloc_psums()
                    for oc in range(NCH):
                        for lb in range(LB):
                            nc.tensor.matmul(
                                psums[oc][:],
                                x_lo[:, lb * P : (lb + 1) * P],
                                w16_sb[lb][:, oc * NF : (oc + 1) * NF],
                                start=lb == 0,
                                stop=False,
                            )
                        for dp in range(GP):
                            nc.tensor.matmul(
                                psums[oc][:],
                                x_hi[
                                    :, dp * 2 * P : (dp + 1) * 2 * P
                                ].rearrange("p (h m) -> p h m", h=2),
                                w8_sb[dp][:]
                                .rearrange("p (h o) -> p h o", h=2)[
                                    :, :, oc * NF : (oc + 1) * NF
                                ],
                                start=False,
                                stop=dp == GP - 1,
                                perf_mode=mybir.MatmulPerfMode.DoubleRow,
                            )
                        evict(opool, mt, psums, ocs=[oc])
    nc.compile()
    return nc


def _prep_inputs(x, weight, bias):
    import ml_dtypes

    f8 = ml_dtypes.float8_e4m3
    x = np.asarray(x, dtype=np.float32)
    weight = np.asarray(weight, dtype=np.float32)
    bias = np.asarray(bias, dtype=np.float32)

    xf = np.ascontiguousarray(x.reshape(M_TOT, D_IN))
    x8 = xf[:, : G * P].astype(f8)
    x16 = xf[:, G * P :].astype(np.float16)

    qw = np.sign(weight)  # [o, d] f32

    # per o-group weights + broadcast bias, shared by cores in the group
    w8_og, w16_og, bias_og = [], [], []
    for og in range(OG):
        o0 = og * O_SH
        blk = np.ascontiguousarray(qw[o0 : o0 + O_SH, :].T)  # [d, o] f32
        # w8[dp, d_in, h*O_SH + o]  (k-blocks [0, G))
        w8 = (
            blk[: G * P]
            .astype(f8)
            .reshape(GP, 2, P, O_SH)
            .transpose(0, 2, 1, 3)
            .reshape(GP, P, 2 * O_SH)
        )
        w8_og.append(np.ascontiguousarray(w8))
        # w16 groups: [d_in, j*O_SH + o] for the 4 k-blocks of the group
        w16b = blk[G * P :].astype(np.float16).reshape(LB, P, O_SH)
        grps, lb0 = [], 0
        for n in WGS:
            grps.append(
                np.ascontiguousarray(
                    w16b[lb0 : lb0 + n].transpose(1, 0, 2)
                ).reshape(P, n * O_SH)
            )
            lb0 += n
        w16_og.append(grps)
        bias_og.append(
            np.ascontiguousarray(
                np.broadcast_to(bias[o0 : o0 + O_SH], (P, O_SH))
            )
        )

    # per m-group x layouts, shared by cores in the group
    xh_mg, xl_mg, xhs_mg, xls_mg = [], [], [], []
    for mg in range(MG):
        m0 = mg * M_SH
        # fp8 steady state: [mt, d, dp*256 + h*128 + m]
        r = x8[m0 : m0 + M_SH].reshape(MT, P, GP, 2, P)  # [mt,m,dp,h,d]
        xh = np.ascontiguousarray(r.transpose(0, 4, 2, 3, 1)).reshape(
            MT, P, G * P
        )
        xh_mg.append(xh)
        # fp16 steady state: [mt, d, lb*128 + m]
        r = x16[m0 : m0 + M_SH].reshape(MT, P, LB, P)  # [mt,m,lb,d]
        xl = np.ascontiguousarray(r.transpose(0, 3, 2, 1)).reshape(
            MT, P, LB * P
        )
        xl_mg.append(xl)
        # startup copies, k-major over the first ST m-tiles, packed with the
        # k-block index outermost in the free dim: [d, (dp|lb)*ST*? + st*? + m]
        xhs = np.empty((GP, ST, P, 2 * P), dtype=f8)
        xls = np.empty((LB, ST, P, P), dtype=np.float16)
        for st in range(ST):
            xhs[:, st] = xh[st].reshape(P, GP, 2 * P).transpose(1, 0, 2)
            xls[:, st] = xl[st].reshape(P, LB, P).transpose(1, 0, 2)
        # -> [P, GP*ST*2P] split (dp 0 | dp 1..) and per-chunk [P, n*ST*P]
        xhs_t = xhs.transpose(2, 0, 1, 3)  # [d, dp, st, 2P]
        xhs_mg.append(
            (
                np.ascontiguousarray(xhs_t[:, :1]).reshape(P, ST * 2 * P),
                np.ascontiguousarray(xhs_t[:, 1:]).reshape(
                    P, (GP - 1) * ST * 2 * P
                ),
            )
        )
        xchunks, lb0 = [], 0
        for n in XCH:
            xchunks.append(
                np.ascontiguousarray(
                    xls[lb0 : lb0 + n].transpose(2, 0, 1, 3)
                ).reshape(P, n * ST * P)
            )
            lb0 += n
        xls_mg.append(xchunks)

    in_maps = []
    for c in range(N_CORES):
        mg, og = c // OG, c % OG
        m = {
            "xh": xh_mg[mg],
            "xl": xl_mg[mg],
            "xhs0": xhs_mg[mg][0],
            "xhs1": xhs_mg[mg][1],
            "w8": w8_og[og],
            "biasb": bias_og[og],
        }
        for ci, xc in enumerate(xls_mg[mg]):
            m[f"xls{ci}"] = xc
        for qi, wg in enumerate(w16_og[og]):
            m[f"w16g{qi}"] = wg
        in_maps.append(m)
    return in_maps


def run(inputs, trace=False):
    """Run the SPMD kernel; returns (full_output, BassKernelResults)."""
    if "nc" not in _CACHE:
        _CACHE["nc"] = _build()
    nc = _CACHE["nc"]
    in_maps = _prep_inputs(inputs["x"], inputs["weight"], inputs["bias"])
    res = run_bass_kernel_spmd(nc, in_maps, list(range(N_CORES)), trace=trace)
    out = np.empty((M_TOT, D_OUT), dtype=np.float32)
    for c in range(N_CORES):
        mg, og = c // OG, c % OG
        out[mg * M_SH : (mg + 1) * M_SH, og * O_SH : (og + 1) * O_SH] = res.results[
            c
        ]["out"]
    return out.reshape(B, S, D_OUT), res


def kernel(x, weight, bias):
    out, _ = run({"x": x, "weight": weight, "bias": bias})
    return out


# revision 34
# speedup vs baseline: 1.0069x; 1.0069x over previous
"""BitNetLinear on 8 Trainium2 NeuronCores.

Computes out = x @ sign(weight).T + bias for x[4,2048,4096] f32,
weight[4096,4096] f32, bias[4096] f32.

Strategy: 2-way data parallel over rows x 4-way tensor parallel over
out_features (each core owns a [4096, 1024] block of the [8192, 4096]
output; no collectives, host stitches blocks).

Per core the contraction (4096 = 32 blocks of 128) is mixed-precision:
  - k-blocks [0, G)   : x quantized e4m3, fp8 DoubleRow matmuls
                        (k=256/instr, ~229 ns measured at N=512);
  - k-blocks [G, 32)  : x in fp16, normal matmuls (213.3 ns at N=512).
sign(weight) is exact in both dtypes; PSUM accumulates fp32. The only
error source is e4m3 quantization of x on the G fp8 blocks: measured
rel-l2 = 2.65e-2*sqrt(G/32) on the benchmark inputs (fp16-only is
2.1e-4), and HW reproduces the numpy prediction to 4 digits. G=12
gives rel-l2 1.63e-2 / scale-relative absmax 1.77e-2 against the 2e-2
gate. A fp8 hi/lo split is dominated by plain fp16 (DoubleRow costs
~1.1x a fp16 matmul per instruction), so raw-fp8 blocks are the only
way below one fp16 pass; the e4m3 error bound then caps G.

Layouts are precomputed on the host so every DMA is contiguous, with
startup tensors packed into few wide-line transfers (2KB-per-partition
lines cap core DMA at ~235 GB/s; 3-8KB lines reach ~340 GB/s). All
weights stay resident in SBUF, x tiles stream per m-tile, and each
[128, 512] output chunk accumulates (32-G) fp16 + G/2 DoubleRow
matmuls before a DVE eviction fused with the bias add (eviction DMAs
ride the scalar queue, x loads the sync queue). The first three
m-tiles run jointly, k-block-major, so PE consumption paces the
weight preload instead of stalling on it. Steady-state m-tiles run in
groups of three with alternating block order (lo,lo,lo,hi,hi,hi |
hi,hi,hi,lo,lo,lo) so fp16<->DoubleRow weight-path mode switches cost
one exposed LDWEIGHTS per three m-tiles; all four 2-bank psum
generations keep the 8 PSUM banks cycling without stalls.
"""

import sys
import types

import numpy as np

import concourse.mybir as mybir
import concourse.tile as tile
from concourse import bacc
from concourse.bass_utils import run_bass_kernel_spmd


def _ensure_axon_hooks():
    """run_bass_kernel_spmd(trace=True) (or BASS_TRACE=1 in the env) imports
    antenv.axon_hooks, which some agent images lack. Provide it, and register
    the ctypes NTFF hook if the boot shim is available, so tracing works (or
    degrades to a warning) instead of crashing."""
    try:
        import antenv.axon_hooks  # noqa: F401

        return
    except ImportError:
        pass
    m = types.ModuleType("antenv.axon_hooks")
    m._h = None
    m.set_axon_ntff_profile_hook = lambda h: setattr(m, "_h", h)
    m.get_axon_ntff_profile_hook = lambda: m._h
    sys.modules["antenv.axon_hooks"] = m
    try:
        import antenv

        antenv.axon_hooks = m
    except ImportError:
        pass
    try:
        from trn_agent_boot.trn_boot import _ntff_profile_via_ctypes

        m.set_axon_ntff_profile_hook(
            _ntff_profile_via_ctypes("/opt/axon/libaxon_pjrt.so")
        )
    except Exception:
        pass


_ensure_axon_hooks()

B, S, D_IN, D_OUT = 4, 2048, 4096, 4096
M_TOT = B * S  # 8192
N_CORES = 8
MG, OG = 2, 4  # data-parallel row groups x tensor-parallel out_feature groups
M_SH = M_TOT // MG  # 4096 rows per core
O_SH = D_OUT // OG  # 1024 out features per core
P = 128
DB = D_IN // P  # 32 contraction blocks of 128
G = 12  # k-blocks [0, G) in e4m3 DoubleRow; must be even
GP = G // 2  # fp8 contraction pairs of 256 (DoubleRow)
LB = DB - G  # fp16 contraction blocks
MT = M_SH // P  # 32 m-tiles per core
NF = 512  # moving free dim per matmul (one PSUM bank of fp32)
NCH = O_SH // NF  # 2 output chunks per m-tile
ST = 3  # m-tiles processed jointly in the startup phase
# startup stream granularity: larger per-partition DMA lines lift the
# per-packet-bound DMA rate (2KB lines measured ~235 GB/s core-wide;
# the startup needs ~280)
XCH = [min(8, LB - 8 * i) for i in range((LB + 7) // 8)]  # xls chunks
# w16 groups: first two are pairs so the fp8->fp16 handoff in the startup
# stream doesn't wait on a full 1MB group
WGS = [2, 2] + [4] * ((LB - 4) // 4)
assert sum(WGS) == LB
_WOFF = [sum(WGS[:i]) for i in range(len(WGS))]  # first lb of each group

_CACHE = {}


def _build():
    nc = bacc.Bacc("TRN2", target_bir_lowering=False, debug=False)
    f8, f16, f32 = mybir.dt.float8e4, mybir.dt.float16, mybir.dt.float32

    # steady-state x, one m-tile per row: free = dp*256 + h*128 + m (fp8)
    # and lb*128 + m (fp16)
    xh_d = nc.dram_tensor("xh", [MT, P, G * P], f8, kind="ExternalInput")
    xl_d = nc.dram_tensor("xl", [MT, P, LB * P], f16, kind="ExternalInput")
    # startup copies of m-tiles 0..ST-1, k-block-major: free dim runs over
    # (dp|lb, st, m) so each transfer covers many k-blocks in one DMA with
    # wide per-partition lines
    # dp 0's x and weights ship fused in one wide-line tensor so the very
    # first matmul waits on a single ~0.36 MB transfer of 2.75KB lines
    xw0_d = nc.dram_tensor(
        "xw0", [P, ST * 2 * P + 2 * O_SH], f8, kind="ExternalInput"
    )
    xhs1_d = nc.dram_tensor(
        "xhs1", [P, (GP - 1) * ST * 2 * P], f8, kind="ExternalInput"
    )
    xls_d = [
        nc.dram_tensor(f"xls{c}", [P, n * ST * P], f16, kind="ExternalInput")
        for c, n in enumerate(XCH)
    ]
    # weights: fp8 pair layout per dp (dp 0 rides in xw0), fp16 in groups
    w8_d = nc.dram_tensor(
        "w8", [GP - 1, P, 2 * O_SH], f8, kind="ExternalInput"
    )
    w16_d = [
        nc.dram_tensor(f"w16g{q}", [P, n * O_SH], f16, kind="ExternalInput")
        for q, n in enumerate(WGS)
    ]
    bias_d = nc.dram_tensor("biasb", [P, O_SH], f32, kind="ExternalInput")
    out_d = nc.dram_tensor("out", [M_SH, O_SH], f32, kind="ExternalOutput")

    with tile.TileContext(nc) as tc:
        with (
            tc.tile_pool(name="wpool", bufs=1) as wpool,
            tc.tile_pool(name="xpool", bufs=6) as xpool,
            tc.tile_pool(name="psum", bufs=4, space="PSUM") as psum_pool,
        ):

            def load_x(mt):
                x_hi = xpool.tile([P, G * P], f8, name="x_hi", tag="xhi")
                x_lo = xpool.tile([P, LB * P], f16, name="x_lo", tag="xlo")
                nc.sync.dma_start(out=x_hi[:], in_=xh_d[mt])
                nc.sync.dma_start(out=x_lo[:], in_=xl_d[mt])
                return x_hi, x_lo

            def alloc_psums():
                return [
                    psum_pool.tile([P, NF], f32, name=f"ps{oc}", tag=f"ps{oc}")
                    for oc in range(NCH)
                ]

            def lo_block(x_lo, psums, opens, closes):
                # full fp16 pass over one m-tile; opens/closes the psum
                # accumulation group if it is the first/last block issued
                for lb in range(LB):
                    for oc in range(NCH):
                        nc.tensor.matmul(
                            psums[oc][:],
                            x_lo[:, lb * P : (lb + 1) * P],
                            w16_sb[lb][:, oc * NF : (oc + 1) * NF],
                            start=opens and lb == 0,
                            stop=closes and lb == LB - 1,
                        )

            def hi_block(x_hi, psums, opens, closes):
                # full DoubleRow fp8 pass over one m-tile
                for dp in range(GP):
                    lhsT3 = x_hi[:, dp * 2 * P : (dp + 1) * 2 * P].rearrange(
                        "p (h m) -> p h m", h=2
                    )
                    for oc in range(NCH):
                        nc.tensor.matmul(
                            psums[oc][:],
                            lhsT3,
                            w8_sb[dp]
                            .rearrange("p (h o) -> p h o", h=2)[
                                :, :, oc * NF : (oc + 1) * NF
                            ],
                            start=opens and dp == 0,
                            stop=closes and dp == GP - 1,
                            perf_mode=mybir.MatmulPerfMode.DoubleRow,
                        )

            def evict(opool, mt, psums, ocs=None):
                for oc in ocs if ocs is not None else range(NCH):
                    o_sb = opool.tile([P, NF], f32, name="o_sb", tag=f"o{oc}")
                    nc.vector.tensor_add(
                        o_sb[:], psums[oc][:], bias_sb[:, oc * NF : (oc + 1) * NF]
                    )
                    # scalar queue: keeps evictions off the sync queue so
                    # steady x loads never wait behind them
                    nc.scalar.dma_start(
                        out=out_d[mt * P : (mt + 1) * P, oc * NF : (oc + 1) * NF],
                        in_=o_sb[:],
                    )

            w8_sb = []
            w16_sb = []
            with tc.tile_pool(name="xstart", bufs=1) as xstart_pool:
                # startup x (m-tiles 0..ST-1) in k-major order plus the
                # weight stream, interleaved in consumption order so each
                # tile lands as the PE needs it: fp8 phase first, then the
                # fp16 blocks (xls chunk / w16 group issued just before the
                # k-blocks they cover)
                xw0 = wpool.tile(
                    [P, ST * 2 * P + 2 * O_SH], f8, name="xw0", tag="xw0"
                )
                nc.sync.dma_start(out=xw0[:], in_=xw0_d[:])
                xhs0_sb = xw0[:, : ST * 2 * P]
                w8_sb.append(xw0[:, ST * 2 * P :])
                xhs1_sb = xstart_pool.tile(
                    [P, (GP - 1) * ST * 2 * P], f8, name="xhs1", tag="xhs1"
                )
                nc.sync.dma_start(out=xhs1_sb[:], in_=xhs1_d[:])
                for dp in range(1, GP):
                    w8 = wpool.tile(
                        [P, 2 * O_SH], f8, name=f"w8_{dp}", tag=f"w8_{dp}"
                    )
                    nc.sync.dma_start(out=w8[:], in_=w8_d[dp - 1])
                    w8_sb.append(w8[:])
                xls_view = []  # per lb: AP covering [P, ST*P]
                q = -1
                for lb in range(LB):
                    if lb % 8 == 0:
                        c = lb // 8
                        xc = xstart_pool.tile(
                            [P, XCH[c] * ST * P], f16, name=f"xls{c}",
                            tag=f"xls{c}",
                        )
                        nc.sync.dma_start(out=xc[:], in_=xls_d[c][:])
                    if q + 1 < len(WGS) and lb == _WOFF[q + 1]:
                        q += 1
                        wg = wpool.tile(
                            [P, WGS[q] * O_SH], f16, name=f"w16g{q}",
                            tag=f"w16g{q}",
                        )
                        nc.sync.dma_start(out=wg[:], in_=w16_d[q][:])
                    xls_view.append(
                        xc[:, (lb % 8) * ST * P : (lb % 8 + 1) * ST * P]
                    )
                    j = lb - _WOFF[q]
                    w16_sb.append(wg[:, j * O_SH : (j + 1) * O_SH])
                bias_sb = wpool.tile([P, O_SH], f32, name="bias_sb")
                nc.sync.dma_start(out=bias_sb[:], in_=bias_d[:])

                # prefetch steady-state x ahead of the startup evictions
                # (in-order sync stream: later dma_starts would head-of-line
                # block behind eviction DMAs otherwise)
                x_next = {mt: load_x(mt) for mt in (ST, ST + 1)}

                # startup: ST m-tiles jointly, k-major, paced by the weight
                # stream
                psums_st = [alloc_psums() for _ in range(ST)]
                for dp in range(GP):
                    src = xhs0_sb if dp == 0 else xhs1_sb
                    base = 0 if dp == 0 else (dp - 1) * ST
                    for st in range(ST):
                        xh3 = src[
                            :, (base + st) * 2 * P : (base + st + 1) * 2 * P
                        ].rearrange("p (h m) -> p h m", h=2)
                        for oc in range(NCH):
                            nc.tensor.matmul(
                                psums_st[st][oc][:],
                                xh3,
                                w8_sb[dp]
                                .rearrange("p (h o) -> p h o", h=2)[
                                    :, :, oc * NF : (oc + 1) * NF
                                ],
                                start=dp == 0,
                                stop=False,
                                perf_mode=mybir.MatmulPerfMode.DoubleRow,
                            )
                for lb in range(LB):
                    for st in range(ST):
                        for oc in range(NCH):
                            nc.tensor.matmul(
                                psums_st[st][oc][:],
                                xls_view[lb][:, st * P : (st + 1) * P],
                                w16_sb[lb][:, oc * NF : (oc + 1) * NF],
                                start=False,
                                stop=lb == LB - 1,
                            )

            with tc.tile_pool(name="opool", bufs=2) as opool:
                for st in range(ST):
                    evict(opool, st, psums_st[st])

                # Steady state: groups of m-tiles with alternating block order
                # (lo,..,hi,.. | hi,..,lo,.. | ...) so fp16<->DoubleRow
                # weight-path mode switches drop to one per group (the group
                # boundary joins identical modes). The startup ends on a fp16
                # matmul, so the first group opens lo. Group size 3 holds
                # 3 psum gens (6 banks) live, within the 4-gen pool.
                # First group is a pair: its 2nd psum gen recycles a startup
                # gen, and the smaller group keeps that wait off the critical
                # path right at the transition.
                sizes = [2] + [3] * ((MT - 1 - ST - 4) // 3) + [2]
                assert sum(sizes) == MT - 1 - ST
                groups = []
                t = ST
                for n in sizes:
                    groups.append(tuple(range(t, t + n)))
                    t += n
                for pi_, grp in enumerate(groups):
                    xs = [
                        x_next.pop(m) if m in x_next else load_x(m)
                        for m in grp
                    ]
                    pss = [alloc_psums() for _ in grp]
                    ii = range(len(grp))
                    if pi_ % 2 == 0:
                        for i in ii:
                            lo_block(xs[i][1], pss[i], True, False)
                        for i in ii:
                            hi_block(xs[i][0], pss[i], False, True)
                    else:
                        for i in ii:
                            hi_block(xs[i][0], pss[i], True, False)
                        for i in ii:
                            lo_block(xs[i][1], pss[i], False, True)
                    for i in ii:
                        evict(opool, grp[i], pss[i])
                for mt in (MT - 1,):
                    # last m-tile: oc-major so each output chunk finishes
                    # and evicts as early as possible
                    x_pair = x_next.pop(mt) if mt in x_next else load_x(mt)
                    x_hi, x_lo = x_pair
                    psums = alloc_psums()
                    for oc in range(NCH):
                        for lb in range(LB):
                            nc.tensor.matmul(
                                psums[oc][:],
                                x_lo[:, lb * P : (lb + 1) * P],
                                w16_sb[lb][:, oc * NF : (oc + 1) * NF],
                                start=lb == 0,
                                stop=False,
                            )
                        for dp in range(GP):
                            nc.tensor.matmul(
                                psums[oc][:],
                                x_hi[
                                    :, dp * 2 * P : (dp + 1) * 2 * P
                                ].rearrange("p (h m) -> p h m", h=2),
                                w8_sb[dp]
                                .rearrange("p (h o) -> p h o", h=2)[
                                    :, :, oc * NF : (oc + 1) * NF
                                ],
                                start=False,
                                stop=dp == GP - 1,
                                perf_mode=mybir.MatmulPerfMode.DoubleRow,
                            )
                        evict(opool, mt, psums, ocs=[oc])
    nc.compile()
    return nc


def _prep_inputs(x, weight, bias):
    import ml_dtypes

    f8 = ml_dtypes.float8_e4m3
    x = np.asarray(x, dtype=np.float32)
    weight = np.asarray(weight, dtype=np.float32)
    bias = np.asarray(bias, dtype=np.float32)

    xf = np.ascontiguousarray(x.reshape(M_TOT, D_IN))
    x8 = xf[:, : G * P].astype(f8)
    x16 = xf[:, G * P :].astype(np.float16)

    qw = np.sign(weight)  # [o, d] f32

    # per o-group weights + broadcast bias, shared by cores in the group
    w8_og, w16_og, bias_og = [], [], []
    for og in range(OG):
        o0 = og * O_SH
        blk = np.ascontiguousarray(qw[o0 : o0 + O_SH, :].T)  # [d, o] f32
        # w8[dp, d_in, h*O_SH + o]  (k-blocks [0, G))
        w8 = (
            blk[: G * P]
            .astype(f8)
            .reshape(GP, 2, P, O_SH)
            .transpose(0, 2, 1, 3)
            .reshape(GP, P, 2 * O_SH)
        )
        w8_og.append(np.ascontiguousarray(w8))
        # w16 groups: [d_in, j*O_SH + o] for the 4 k-blocks of the group
        w16b = blk[G * P :].astype(np.float16).reshape(LB, P, O_SH)
        grps, lb0 = [], 0
        for n in WGS:
            grps.append(
                np.ascontiguousarray(
                    w16b[lb0 : lb0 + n].transpose(1, 0, 2)
                ).reshape(P, n * O_SH)
            )
            lb0 += n
        w16_og.append(grps)
        bias_og.append(
            np.ascontiguousarray(
                np.broadcast_to(bias[o0 : o0 + O_SH], (P, O_SH))
            )
        )

    # per m-group x layouts, shared by cores in the group
    xh_mg, xl_mg, xhs_mg, xls_mg = [], [], [], []
    for mg in range(MG):
        m0 = mg * M_SH
        # fp8 steady state: [mt, d, dp*256 + h*128 + m]
        r = x8[m0 : m0 + M_SH].reshape(MT, P, GP, 2, P)  # [mt,m,dp,h,d]
        xh = np.ascontiguousarray(r.transpose(0, 4, 2, 3, 1)).reshape(
            MT, P, G * P
        )
        xh_mg.append(xh)
        # fp16 steady state: [mt, d, lb*128 + m]
        r = x16[m0 : m0 + M_SH].reshape(MT, P, LB, P)  # [mt,m,lb,d]
        xl = np.ascontiguousarray(r.transpose(0, 3, 2, 1)).reshape(
            MT, P, LB * P
        )
        xl_mg.append(xl)
        # startup copies, k-major over the first ST m-tiles, packed with the
        # k-block index outermost in the free dim: [d, (dp|lb)*ST*? + st*? + m]
        xhs = np.empty((GP, ST, P, 2 * P), dtype=f8)
        xls = np.empty((LB, ST, P, P), dtype=np.float16)
        for st in range(ST):
            xhs[:, st] = xh[st].reshape(P, GP, 2 * P).transpose(1, 0, 2)
            xls[:, st] = xl[st].reshape(P, LB, P).transpose(1, 0, 2)
        # -> [P, GP*ST*2P] split (dp 0 | dp 1..) and per-chunk [P, n*ST*P]
        xhs_t = xhs.transpose(2, 0, 1, 3)  # [d, dp, st, 2P]
        xhs_mg.append(
            (
                np.ascontiguousarray(xhs_t[:, :1]).reshape(P, ST * 2 * P),
                np.ascontiguousarray(xhs_t[:, 1:]).reshape(
                    P, (GP - 1) * ST * 2 * P
                ),
            )
        )
        xchunks, lb0 = [], 0
        for n in XCH:
            xchunks.append(
                np.ascontiguousarray(
                    xls[lb0 : lb0 + n].transpose(2, 0, 1, 3)
                ).reshape(P, n * ST * P)
            )
            lb0 += n
        xls_mg.append(xchunks)

    in_maps = []
    for c in range(N_CORES):
        mg, og = c // OG, c % OG
        m = {
            "xh": xh_mg[mg],
            "xl": xl_mg[mg],
            "xw0": np.ascontiguousarray(
                np.concatenate([xhs_mg[mg][0], w8_og[og][0]], axis=1)
            ),
            "xhs1": xhs_mg[mg][1],
            "w8": np.ascontiguousarray(w8_og[og][1:]),
            "biasb": bias_og[og],
        }
        for ci, xc in enumerate(xls_mg[mg]):
            m[f"xls{ci}"] = xc
        for qi, wg in enumerate(w16_og[og]):
            m[f"w16g{qi}"] = wg
        in_maps.append(m)
    return in_maps


def run(inputs, trace=False):
    """Run the SPMD kernel; returns (full_output, BassKernelResults)."""
    if "nc" not in _CACHE:
        _CACHE["nc"] = _build()
    nc = _CACHE["nc"]
    in_maps = _prep_inputs(inputs["x"], inputs["weight"], inputs["bias"])
    res = run_bass_kernel_spmd(nc, in_maps, list(range(N_CORES)), trace=trace)
    out = np.empty((M_TOT, D_OUT), dtype=np.float32)
    for c in range(N_CORES):
        mg, og = c // OG, c % OG
        out[mg * M_SH : (mg + 1) * M_SH, og * O_SH : (og + 1) * O_SH] = res.results[
            c
        ]["out"]
    return out.reshape(B, S, D_OUT), res


def kernel(x, weight, bias):
    out, _ = run({"x": x, "weight": weight, "bias": bias})
    return out
